# revision 4
# baseline (speedup 1.0000x reference)
"""Trainium2 Bass kernel for nn_EstimateGrassmann.

Math: for each sample b with z = 1-x (indicator of zeros),
  m_b = sigma @ diag(2x-1) + diag(1-x)  and  |det(m_b)| = |det(sigma - diag(z))|.
Since p_b = det(m_b) > 0, log p_b = sum_k log|pivot_k| of an unpivoted
Gaussian elimination of mtilde = sigma - diag(z).  mtilde is strongly
diagonally dominant in magnitude (pivots stay in ~[0.46, 0.54]), so no
pivoting is needed.

Layout: 128 samples per partition-tile, the 32x32 matrix flattened along
the free dimension; G tiles are eliminated together so each elimination
step is 4 wide DVE instructions (reciprocal, column scale, broadcast
outer product, subtract) covering G*128 samples via multi-dim access
patterns with stride-0 broadcasts.  log|pivot| = 0.5*Ln(pivot^2): Square
and Ln ride the scalar engine, the per-tile reduction on the vector
engine.

sigma = inv(B_ @ inv(C_) + I) is computed on-device; inverses via Newton
iteration X <- X(2I - AX) on the tensor engine, carrying X^T through the
iteration so no per-step transposes are needed.  stabilize() makes both
matrices strongly row-diagonally dominant, so X0 = alpha*I with a fixed
alpha below the guaranteed 1/maxabsrowsum bound converges quadratically.

Sharding: pure data parallel over the batch (65536/8 = 8192 samples per
core; B, C replicated).  Each core returns [128, NTILES] partial sums of
Ln(pivot^2); the host all-reduces with a float64 sum * 0.5 / BATCH.
"""

import numpy as np

DIM = 32
BATCH = 65536
NCORES = 8
P = 128
SHARD = BATCH // NCORES          # 8192
NTILES_FULL = SHARD // P         # 64
GROUP = 8

ALPHA_C = 1.0 / 4.0
ALPHA_LAM = 1.0 / 5.0
NEWTON_ITERS = 8

_cache = {}


def _build(ntiles, group):
    import concourse.bass as bass
    import concourse.mybir as mybir
    from concourse.tile import TileContext

    fp32 = mybir.dt.float32
    i32 = mybir.dt.int32
    AF = mybir.ActivationFunctionType
    OP = mybir.AluOpType
    AX = mybir.AxisListType

    G = min(group, ntiles)
    assert ntiles % G == 0
    ngroups = ntiles // G
    nshard = ntiles * P
    nc = bass.Bass()
    x_d = nc.dram_tensor("x", [nshard, DIM], i32, kind="ExternalInput")
    b_d = nc.dram_tensor("B", [DIM, DIM], fp32, kind="ExternalInput")
    c_d = nc.dram_tensor("C", [DIM, DIM], fp32, kind="ExternalInput")
    eye_d = nc.dram_tensor("eye", [DIM, DIM], fp32, kind="ExternalInput")
    out_d = nc.dram_tensor("out", [P, ntiles], fp32, kind="ExternalOutput")

    with TileContext(nc) as tc:
        with tc.tile_pool(name="const", bufs=1) as cpool, \
             tc.tile_pool(name="setup", bufs=1) as spool, \
             tc.tile_pool(name="psum", bufs=2, space="PSUM") as qpool, \
             tc.tile_pool(name="dram", bufs=1, space="DRAM") as dpool, \
             tc.tile_pool(name="big", bufs=1) as gpool, \
             tc.tile_pool(name="m", bufs=2) as mpool, \
             tc.tile_pool(name="t", bufs=2) as tpool, \
             tc.tile_pool(name="cs", bufs=2) as cspool, \
             tc.tile_pool(name="r", bufs=4) as rpool, \
             tc.tile_pool(name="d2", bufs=2) as d2pool:

            eye = cpool.tile([DIM, DIM], fp32, name="eye_sb")
            ome = cpool.tile([DIM, DIM], fp32, name="ome_sb")
            eye2 = cpool.tile([DIM, DIM], fp32, name="eye2_sb")
            nc.sync.dma_start(eye[:], eye_d[:])
            # derive 1-eye and 2*eye on device (fewer DMA sems to wait on)
            nc.vector.tensor_scalar(ome[:], eye[:], -1.0, 1.0,
                                    op0=OP.mult, op1=OP.add)
            nc.vector.tensor_scalar(eye2[:], eye[:], 2.0, None, op0=OP.mult)

            def stabilize(m_sb, nm):
                off = spool.tile([DIM, DIM], fp32, name=f"off_{nm}")
                rl = spool.tile([DIM, DIM], fp32, name=f"rl_{nm}")
                ab = spool.tile([DIM, DIM], fp32, name=f"ab_{nm}")
                ab2 = spool.tile([DIM, DIM], fp32, name=f"ab2_{nm}")
                rs = spool.tile([DIM, 1], fp32, name=f"rs_{nm}")
                rs2 = spool.tile([DIM, 1], fp32, name=f"rs2_{nm}")
                st = spool.tile([DIM, DIM], fp32, name=f"st_{nm}")
                nc.vector.tensor_mul(off[:], m_sb[:], ome[:])
                nc.scalar.activation(rl[:], m_sb[:], AF.Relu)
                # diag of stabilized matrix = sum_j |off_ij| + relu(M_ii)
                nc.scalar.activation(ab[:], off[:], AF.Abs)
                nc.vector.tensor_mul(ab2[:], rl[:], eye[:])
                nc.vector.tensor_reduce(rs[:], ab[:], axis=AX.X, op=OP.add)
                nc.vector.tensor_reduce(rs2[:], ab2[:], axis=AX.X, op=OP.add)
                nc.vector.tensor_add(rs[:], rs[:], rs2[:])
                nc.vector.scalar_tensor_tensor(
                    st[:], eye[:], rs[:, 0:1], off[:], op0=OP.mult, op1=OP.add)
                return st

            def transpose32(src, nm):
                ps = qpool.tile([DIM, DIM], fp32, name=f"pt_{nm}", tag="ps")
                dst = spool.tile([DIM, DIM], fp32, name=f"tr_{nm}")
                nc.tensor.transpose(ps[:], src[:], eye[:])
                nc.scalar.copy(dst[:], ps[:])
                return dst

            def inv32(a_sb, alpha, nm):
                at = transpose32(a_sb, nm)
                xx = spool.tile([DIM, DIM], fp32, name=f"x_{nm}")
                xt = spool.tile([DIM, DIM], fp32, name=f"xt_{nm}")
                nc.vector.tensor_scalar(xx[:], eye[:], alpha, None, op0=OP.mult)
                nc.vector.tensor_scalar(xt[:], eye[:], alpha, None, op0=OP.mult)
                for it in range(NEWTON_ITERS):
                    t1 = qpool.tile([DIM, DIM], fp32, name=f"nt_{nm}_{it}", tag="ps")
                    nc.tensor.matmul(t1[:], at[:], xx[:], start=True, stop=True)
                    w = spool.tile([DIM, DIM], fp32, name=f"w_{nm}_{it}", tag="w")
                    nc.vector.scalar_tensor_tensor(
                        w[:], t1[:], -1.0, eye2[:], op0=OP.mult, op1=OP.add)
                    x2 = qpool.tile([DIM, DIM], fp32, name=f"x2_{nm}_{it}", tag="ps2")
                    xt2 = qpool.tile([DIM, DIM], fp32, name=f"xt2_{nm}_{it}", tag="ps3")
                    nc.tensor.matmul(x2[:], xt[:], w[:], start=True, stop=True)
                    nc.tensor.matmul(xt2[:], w[:], xt[:], start=True, stop=True)
                    nc.scalar.copy(xx[:], x2[:])
                    nc.scalar.copy(xt[:], xt2[:])
                return xx, xt

            b_sb = spool.tile([DIM, DIM], fp32, name="b_sb")
            c_sb = spool.tile([DIM, DIM], fp32, name="c_sb")
            nc.sync.dma_start(b_sb[:], b_d[:])
            nc.sync.dma_start(c_sb[:], c_d[:])
            bs = stabilize(b_sb, "b")
            cs = stabilize(c_sb, "c")
            invc, _ = inv32(cs, ALPHA_C, "c")
            bt = transpose32(bs, "bt")
            lamp = qpool.tile([DIM, DIM], fp32, name="lamp", tag="ps")
            nc.tensor.matmul(lamp[:], bt[:], invc[:], start=True, stop=True)
            lam = spool.tile([DIM, DIM], fp32, name="lam")
            nc.vector.tensor_add(lam[:], lamp[:], eye[:])
            sigma, _ = inv32(lam, ALPHA_LAM, "s")

            # replicate sigma to all 128 partitions via a DRAM round trip
            sig_dram = dpool.tile([DIM, DIM], fp32, name="sig_dram")
            nc.sync.dma_start(sig_dram[:], sigma[:])
            sig_rep = gpool.tile([P, DIM * DIM], fp32, name="sig_rep")
            src = sig_dram[:].rearrange("a b -> (a b)").unsqueeze(0) \
                             .broadcast_to([P, DIM * DIM])
            nc.sync.dma_start(sig_rep[:], src)

            # load x as [128, ntiles, 32]; zf = 1 - x (fp32)
            xi = gpool.tile([P, ntiles * DIM], i32, name="xi")
            xv = x_d[:].rearrange("(t p) d -> p t d", p=P)
            nc.sync.dma_start(xi[:].rearrange("p (t d) -> p t d", d=DIM), xv)
            zf = gpool.tile([P, ntiles * DIM], fp32, name="zf")
            nc.vector.tensor_scalar(zf[:], xi[:], -1.0, 1.0,
                                    op0=OP.mult, op1=OP.add)
            zf3 = zf[:].rearrange("p (t d) -> p t d", d=DIM)

            partials = gpool.tile([P, ntiles], fp32, name="partials")
            sig_bc = sig_rep[:].unsqueeze(1).broadcast_to([P, G, DIM * DIM])

            for g in range(ngroups):
                m = mpool.tile([P, G * DIM * DIM], fp32, name=f"m_{g}", tag="m")
                m3 = m.rearrange("p (g f) -> p g f", f=DIM * DIM)
                mv = m.rearrange("p (g i j) -> p g i j", i=DIM, j=DIM)
                nc.gpsimd.tensor_copy(m3, sig_bc)
                dview = m3[:, :, 0:DIM * DIM:DIM + 1]          # [P, G, 32]
                nc.vector.tensor_sub(dview, dview,
                                     zf3[:, g * G:(g + 1) * G, :])
                rg = rpool.tile([P, G], fp32, name=f"r_{g}", tag="r")
                csg = cspool.tile([P, G * DIM], fp32, name=f"cs_{g}", tag="cs")
                cs3 = csg.rearrange("p (g i) -> p g i", g=G)
                for k in range(DIM - 1):
                    n = DIM - 1 - k
                    nc.vector.reciprocal(rg[:], mv[:, :, k, k])
                    col = mv[:, :, k + 1:, k]                  # [P, G, n]
                    csv = cs3[:, :, :n]
                    rb = rg[:].unsqueeze(2).broadcast_to([P, G, n])
                    nc.vector.tensor_mul(csv, col, rb)
                    tt = tpool.tile([P, G * n * n], fp32, name=f"t_{g}_{k}",
                                    tag="t")
                    tv = tt.rearrange("p (g i j) -> p g i j", i=n, j=n)
                    csb = csv.unsqueeze(3).broadcast_to([P, G, n, n])
                    rowb = mv[:, :, k:k + 1, k + 1:].broadcast_to([P, G, n, n])
                    nc.vector.tensor_mul(tv, csb, rowb)
                    sub = mv[:, :, k + 1:, k + 1:]
                    nc.vector.tensor_sub(sub, sub, tv)
                d2 = d2pool.tile([P, G * DIM], fp32, name=f"d2_{g}", tag="d2")
                nc.scalar.activation(d2[:], dview, AF.Square)
                lnd = d2pool.tile([P, G * DIM], fp32, name=f"lnd_{g}", tag="lnd")
                nc.scalar.activation(lnd[:], d2[:], AF.Ln)
                pview = partials[:, g * G:(g + 1) * G].unsqueeze(2)
                nc.vector.tensor_reduce(
                    pview, lnd[:].rearrange("p (g d) -> p g d", d=DIM),
                    axis=AX.X, op=OP.add)

            nc.sync.dma_start(out_d[:], partials[:])
    return nc


def _get(ntiles, group=GROUP):
    key = (ntiles, group)
    if key not in _cache:
        _cache[key] = _build(ntiles, group)
    return _cache[key]


def _legalize_bir(bir_json: bytes) -> bytes:
    """Walrus here allows only ONE embedded sem wait per instruction; split
    extra waits into standalone EventSemaphore instructions (same engine,
    executed in stream order just before the owning instruction)."""
    import json as _json
    j = _json.loads(bir_json)
    n_split = 0
    for fn in j.get("functions", []):
        for blk in fn.get("blocks", []):
            out = []
            for inst in blk.get("instructions", []):
                si = inst.get("sync_info") or {}
                waits = si.get("on_wait") or []
                if len(waits) > 1:
                    for wi, w in enumerate(waits[:-1]):
                        out.append({
                            "debug": 0,
                            "engine": inst.get("engine", "Unassigned"),
                            "ins": [], "outs": [],
                            "name": f"{inst.get('name','I')}-w{wi}",
                            "opcode": "EventSemaphore",
                            "sync_info": {"on_wait": [w], "on_update": []},
                        })
                        n_split += 1
                    si = dict(si)
                    si["on_wait"] = [waits[-1]]
                    inst = dict(inst)
                    inst["sync_info"] = si
                out.append(inst)
            blk["instructions"] = out
    if n_split:
        print(f"[legalize] split {n_split} extra sem waits")
    return _json.dumps(j).encode()


_patched = False


def _install_patch():
    global _patched
    if _patched:
        return
    import concourse.bass_utils as bu
    import concourse.bass2jax as b2j
    orig = bu.compile_bir_kernel

    def patched(bir_json, tmpdir, neff_name="file.neff"):
        return orig(_legalize_bir(bir_json), tmpdir, neff_name)

    bu.compile_bir_kernel = patched
    b2j.compile_bir_kernel = patched
    _patched = True


def _run(x, B, C, ntiles=NTILES_FULL, ncores=NCORES, group=GROUP, trace=False):
    from concourse.bass_utils import run_bass_kernel_spmd
    _install_patch()

    x = np.ascontiguousarray(np.asarray(x, dtype=np.int32))
    B = np.asarray(B, dtype=np.float32)
    C = np.asarray(C, dtype=np.float32)
    eye = np.eye(DIM, dtype=np.float32)
    nshard = ntiles * P
    nc = _get(ntiles, group)
    in_maps = []
    for c in range(ncores):
        in_maps.append({
            "x": x[c * nshard:(c + 1) * nshard],
            "B": B, "C": C, "eye": eye,
        })
    res = run_bass_kernel_spmd(nc, in_maps, core_ids=list(range(ncores)),
                               trace=trace)
    return res


def kernel(x, B, C):
    res = _run(x, B, C)
    total = 0.0
    for r in res.results:
        total += r["out"].astype(np.float64).sum()
    return np.float32(0.5 * total / BATCH)
